# revision 1
# baseline (speedup 1.0000x reference)
"""Trainium2 Bass kernel for nn_MetaPosterior (loss_fn).

Math
----
Reference computes, per (a, p, k) with A=1024, P=4, K=8, D1=1025:
    theta_p = meta_theta[perm], mk_p = m_ks[k, perm], g_p = grads_v[k, perm]
    S       = sum_{r=2..D1-1} g_p[r] * (theta_p[r] - mk_p[r])
    lp      = sum_{i=0,1} [ -0.5*log(2pi) + 0.5*log(g_p[i])
                            - 0.5*g_p[i]*(theta_p[i] - mk_p[i] + S)^2 ]
(the 1/g_i and outer-product factors in the source cancel exactly).

Because perm is a true permutation of [0, D1), the tail sum telescopes:
    S = T[k] - h[k, i0] - h[k, i1],   h[k, d] = g[k, d]*(theta[d] - m_ks[k, d])
    T[k] = sum_d h[k, d],             i0, i1 = perm[0], perm[1]
so only the first two entries of each permutation are needed.  Expanding the
squares with A[k, d] = 0.5*log(g) - 0.5*g*c^2 (c = theta - m_ks) and
Sneg = h0 + h1 - T[k] = -S:
    lp + log(2pi) = (A0 + A1) + Sneg^2 + T[k]*Sneg - 0.5*(g0 + g1)*Sneg^2
Sum over all (a, p, k): the T[k]*Sneg term reduces to T[k] * sum(Sneg) per
partition, so the device only accumulates four per-partition totals:
sum(Sneg), sum(Sneg^2), sum(-0.5*(g0+g1)*Sneg^2), sum(A); the host combines.

Kernel (raw Bass, per core)
---------------------------
Sharding: leading 'a' axis of perms split across the 8 NeuronCores (128
a-values -> 4096 (a,p,k) pairs -> 8192 gather indices per core); the small
(K, D1) tables are replicated, packed as 256B-strided HBM rows indexed by
k*1025 + j with fields [g, h, A].
  1. one DMA brings in the 8192 int16 indices (+T[k] packed in 2 tail cols),
  2. one SWDGE dma_gather fetches 128B of each indexed row (the 256B-row
     stride keeps descriptors legal; 128B payload benches fastest),
  3. five Vector-engine instructions compute Sneg, Sneg^2, GS, and the
     accum_out reductions (per-instruction overhead dominates on this stack,
     so everything stays on one engine with a single semaphore hop),
  4. one DMA writes the [128, 4] partials; host reduces in f64, adds the
     constant + prior terms, negates.
"""

import numpy as np

import concourse.bacc as bacc
import concourse.mybir as mybir
from concourse.bass_utils import run_bass_kernel_spmd

LOG2PI = float(np.log(2.0 * np.pi))
DIM, K, P, M_COND = 1024, 8, 4, 2
D1 = DIM + 1                      # 1025
N_CORES = 8
A_PER_CORE = DIM // N_CORES       # 128
TRIPLES = A_PER_CORE * P * K      # 4096 (a', p, k) pairs per core
N_IDX = 2 * TRIPLES               # 8192 gather indices per core
CHUNKS = N_IDX // 128             # 64 chunks in the gathered tile
HALF = CHUNKS // 2                # slot-0 chunks 0..31, slot-1 chunks 32..63
ROW = 64                          # table row stride: 64 f32 = 256 B
ESZ = 32                          # gathered payload: 32 f32 = 128 B
TBL_ROWS = K * D1                 # 8200 combined (k, j) rows
IDXC = N_IDX // 16                # 512 idx columns
IDX_COLS = IDXC + 2               # + 2 int16 columns carrying T[k] (bitcast)

_PROGS = {}  # iters -> compiled program (built once per process)


def _dma_gather_thin(gp, out_ap, in_ap, idxs_ap, num_idxs, elem_size, elem_step):
    """dma_gather with elem_size_bytes not a multiple of 256.  bass asserts
    the 256B granularity (a transpose-mode restriction) on the wrapper, so
    emit InstDMAGatherAnt directly; the row stride (elem_step) must still be
    a multiple of 256B."""
    stride_bytes_256 = (elem_step * 4) // 256
    _in_ap = gp.lower_ap_dma(in_ap, for_custom_bir_dma=True)
    _idxs_ap = gp.lower_ap(idxs_ap)
    _out_ap = gp.lower_ap(out_ap)
    return gp.add_instruction(mybir.InstDMAGatherAnt(
        name=gp.bass.get_next_instruction_name(),
        ins=[*_in_ap, _idxs_ap, gp.lower_val_access(gp.to_reg(num_idxs))],
        outs=[_out_ap],
        transpose=False, num_idxs=num_idxs, elem_size=elem_size,
        stride_bytes_256=stride_bytes_256, gen_mode=0, single_packet=False,
        queue_num=0, sbuf_tokens_per_rank=0, sbuf_free_dim_per_rank=0,
        sbuf_free_dim_pad_per_rank=0, sbuf_byte_offset=0))


def _build_program(iters=1):
    f32, i16 = mybir.dt.float32, mybir.dt.int16
    alu = mybir.AluOpType
    nc = bacc.Bacc("TRN2")

    tbl = nc.dram_tensor("tbl", [TBL_ROWS, ROW], f32, kind="ExternalInput")
    idx = nc.dram_tensor("idx", [128, IDX_COLS], i16, kind="ExternalInput")
    out = nc.dram_tensor("out", [128, 4], f32, kind="ExternalOutput")

    with (
        nc.sbuf_tensor("idx_sb", [128, IDX_COLS], i16) as idx_sb,
        nc.sbuf_tensor("gath", [128, CHUNKS, ESZ], f32) as gath,
        nc.sbuf_tensor("sneg", [128, HALF], f32) as sneg,
        nc.sbuf_tensor("s2", [128, HALF], f32) as s2,
        nc.sbuf_tensor("gs", [128, HALF], f32) as gs,
        nc.sbuf_tensor("j1", [128, HALF], f32) as j1,
        nc.sbuf_tensor("j2", [128, CHUNKS], f32) as j2,
        nc.sbuf_tensor("red", [128, 4], f32) as red,
        nc.semaphore("s_in") as s_in,
        nc.semaphore("s_g") as s_g,
        nc.semaphore("s_v") as s_v,
        nc.semaphore("s_o") as s_o,
        nc.Block() as block,
    ):
        tk = idx_sb[:, IDXC : IDXC + 2].bitcast(f32)      # [128, 1] T[k]
        g0 = gath[:, 0:HALF, 0]
        g1 = gath[:, HALF:CHUNKS, 0]
        h0 = gath[:, 0:HALF, 1]
        h1 = gath[:, HALF:CHUNKS, 1]
        a_all = gath[:, 0:CHUNKS, 2]

        @block.gpsimd
        def _(gp):
            gp.dma_start(idx_sb[:], idx[:]).then_inc(s_in, 16)
            gp.wait_ge(s_in, 16)
            for i in range(iters):
                if i > 0:  # gath consumers of iter i-1 done before overwrite
                    gp.wait_ge(s_v, i)
                _dma_gather_thin(
                    gp, gath[:], tbl[:, 0:ESZ], idx_sb[:, 0:IDXC],
                    N_IDX, ESZ, ROW,
                ).then_inc(s_g, 16)

        @block.vector
        def _(v):
            for i in range(iters):
                v.wait_ge(s_g, 16 * (i + 1))
                # Sneg = (h0 - T[k]) + h1 ; red0 = sum Sneg
                v.scalar_tensor_tensor(
                    sneg[:], h0, tk, h1, alu.subtract, alu.add,
                    accum_out=red[:, 0:1],
                )
                # S2 = Sneg^2 ; red1 = sum S2
                v.scalar_tensor_tensor(
                    s2[:], sneg[:], 0.0, sneg[:], alu.add, alu.mult,
                    accum_out=red[:, 1:2],
                )
                # GS = g0 + g1
                v.tensor_tensor(gs[:], g0, g1, alu.add)
                # red2 = sum -0.5*GS*Sneg^2
                v.scalar_tensor_tensor(
                    j1[:], gs[:], -0.5, s2[:], alu.mult, alu.mult,
                    accum_out=red[:, 2:3],
                )
                # red3 = sum A over all chunks
                v.tensor_scalar(
                    j2[:], a_all, 0.0, 0.0, alu.add, alu.add,
                    accum_out=red[:, 3:4],
                ).then_inc(s_v, 1)

        @block.sync
        def _(s):
            s.wait_ge(s_v, iters)
            s.dma_start(out[:], red[:]).then_inc(s_o, 16)
            s.wait_ge(s_o, 16)

    nc.finalize()
    return nc


def _get_program(iters=1):
    if iters not in _PROGS:
        _PROGS[iters] = _build_program(iters)
    return _PROGS[iters]


def _device_inputs(meta_theta, m_ks, grads_v, perms):
    """Host prep: tables (O(K*D1)) and per-core index shards."""
    g = np.asarray(grads_v, np.float32)
    c = (np.asarray(meta_theta, np.float32)[None, :] - np.asarray(m_ks, np.float32))
    c = c.astype(np.float32)
    h = (g * c).astype(np.float32)
    lg = (0.5 * np.log(g.astype(np.float64))).astype(np.float32)
    a_f = (lg - np.float32(0.5) * g * c * c).astype(np.float32)
    t_k = h.astype(np.float64).sum(axis=1).astype(np.float32)  # (K,)

    tbl = np.zeros((TBL_ROWS, ROW), np.float32)
    tbl[:, 0] = g.ravel()
    tbl[:, 1] = h.ravel()
    tbl[:, 2] = a_f.ravel()

    tk_col = t_k[np.arange(128) % K].reshape(128, 1).astype(np.float32)
    tk_i16 = tk_col.view(np.int16)  # [128, 2]

    perms01 = np.ascontiguousarray(np.asarray(perms)[:, :, :, :2])  # (A,P,K,2)
    kvec = np.tile(np.arange(K, dtype=np.int64), TRIPLES // K)      # t = (a',p,k)

    in_maps = []
    for core in range(N_CORES):
        sl = perms01[core * A_PER_CORE : (core + 1) * A_PER_CORE]
        sl = sl.reshape(TRIPLES, 2).astype(np.int64)
        comb0 = kvec * D1 + sl[:, 0]
        comb1 = kvec * D1 + sl[:, 1]
        idx_all = np.concatenate([comb0, comb1]).astype(np.int16)   # (N_IDX,)
        # dma_gather unwraps indices as (s p) over the first 16 partitions;
        # replicate across all 8 Q7 core groups.
        idx16 = idx_all.reshape(N_IDX // 16, 16).T                  # [16, 512]
        idx128 = np.tile(idx16, (8, 1))                             # [128, 512]
        idxc = np.ascontiguousarray(
            np.concatenate([idx128, tk_i16], axis=1)                # [128, 514]
        )
        in_maps.append({"tbl": tbl, "idx": idxc})
    return in_maps


def _finalize(partials, t_k, meta_theta, alpha):
    """Combine per-core partial sums with the constant and prior terms.

    partials: (N_CORES, 128, 4) columns [sum Sneg, sum Sneg^2,
    sum -0.5*GS*Sneg^2, sum A].  The T[k]*Sneg term is T[k] (a per-partition
    constant, k = p mod 8) times column 0.
    """
    partials = np.asarray(partials, np.float64)  # (8, 128, 4)
    tkd = np.asarray(t_k, np.float64)[np.arange(128) % K]  # (128,)
    total = float(
        (partials[:, :, 0] * tkd[None, :]).sum()
        + partials[:, :, 1].sum()
        + partials[:, :, 2].sum()
        + partials[:, :, 3].sum()
    )
    sum_lp = total - LOG2PI * (N_CORES * TRIPLES)
    loss_pred = sum_lp / (P * M_COND * K)
    mt = np.asarray(meta_theta, np.float64)
    a = float(alpha)
    lp_prior = -0.5 * (D1 * LOG2PI + D1 * np.log(a) + float(mt @ mt) / a)
    loss = (1.0 - 1.0 / K) * lp_prior + loss_pred
    return np.float32(-loss)


def run_device(in_maps, iters=1, **kwargs):
    nc = _get_program(iters)
    return run_bass_kernel_spmd(nc, in_maps, list(range(N_CORES)), **kwargs)


def kernel(meta_theta, m_ks, grads_v, perms, alpha):
    g = np.asarray(grads_v, np.float32)
    c = (np.asarray(meta_theta, np.float32)[None, :] - np.asarray(m_ks, np.float32))
    h = (g * c.astype(np.float32)).astype(np.float32)
    t_k = h.astype(np.float64).sum(axis=1).astype(np.float32)

    in_maps = _device_inputs(meta_theta, m_ks, grads_v, perms)
    last_err = None
    for _ in range(3):  # retry transient device/runtime hiccups
        try:
            res = run_device(in_maps)
            break
        except Exception as e:  # noqa: BLE001
            last_err = e
    else:
        raise last_err
    partials = np.stack([r["out"] for r in res.results])  # (8, 128, 4)
    return _finalize(partials, t_k, meta_theta, alpha)



# revision 7
# speedup vs baseline: 2.9728x; 2.9728x over previous
"""Trainium2 Bass kernel for nn_MetaPosterior (loss_fn).

Math
----
Reference computes, per (a, p, k) with A=1024, P=4, K=8, D1=1025:
    theta_p = meta_theta[perm], mk_p = m_ks[k, perm], g_p = grads_v[k, perm]
    S       = sum_{r=2..D1-1} g_p[r] * (theta_p[r] - mk_p[r])
    lp      = sum_{i=0,1} [ -0.5*log(2pi) + 0.5*log(g_p[i])
                            - 0.5*g_p[i]*(theta_p[i] - mk_p[i] + S)^2 ]
(the 1/g_i and outer-product factors in the source cancel exactly).

Because perm is a true permutation of [0, D1), the tail sum telescopes:
    S = T[k] - h[k, i0] - h[k, i1],   h[k, d] = g[k, d]*(theta[d] - m_ks[k, d])
    T[k] = sum_d h[k, d],             i0, i1 = perm[0], perm[1]
so only the first two entries of each permutation are needed.  Expanding the
squares with A[k, d] = 0.5*log(g) - 0.5*g*c^2 (c = theta - m_ks) and
Sneg = h0 + h1 - T[k] = -S:
    lp + log(2pi) = (A0 + A1) + Sneg^2 + T[k]*Sneg - 0.5*(g0 + g1)*Sneg^2
Folding T[k] into per-row table fields (indexed by comb = k*1025 + j):
    f0[comb] = h[k, j] - T[k]/2          ->  f0_0 + f0_1     = Sneg
    f1[comb] = g[k, j]                   ->  f1_0 + f1_1     = g0 + g1
    f2[comb] = A[k, j] + T[k]*f0[comb]   ->  sum(f2_0+f2_1)  = sum(A) + T*Sneg
so per triple:  lp + log(2pi) = Sneg^2 - 0.5*(g0+g1)*Sneg^2 + (f2_0 + f2_1).

Kernel (raw Bass, per core)
---------------------------
Sharding: leading 'a' axis of perms split across the 8 NeuronCores (128
a-values -> 4096 (a,p,k) triples -> 8192 gather indices per core).  The
combined (k, j) table [8200 rows x 3 fields, interleaved] is replicated on
every SBUF partition (98.4 KB/partition).  The gather runs ON-CHIP on the
GpSimd engine (InstAPGather): the 8 Q7 cores each gather 1024 indices (their
own 16-partition-wrapped index list) from their partitions' table copies --
no per-index DMA descriptors (the previous dma_gather bottleneck, ~47ns/idx).
  1. one-time DMAs: table [128, 8200, 3] f32, indices [128, 64] i16,
  2. per iteration: one ap_gather (1024 idx/core-group, d=3) -> [128,1024,3],
     pair layout: cols 0..511 = perm[0] gathers, 512..1023 = perm[1],
  3. five Vector-engine ops on [128, 512] strided views compute
     Sneg, Sneg^2, g0+g1, and the accum_out reductions,
  4. one DMA writes the [128, 8] partials; host takes one row per 16-part
     group, reduces in f64, adds the constant + prior terms, negates.
"""

import numpy as np

import concourse.bacc as bacc
import concourse.mybir as mybir
from concourse.bass_utils import run_bass_kernel_spmd

LOG2PI = float(np.log(2.0 * np.pi))
DIM, K, P, M_COND = 1024, 8, 4, 2
D1 = DIM + 1                      # 1025
N_CORES = 8
A_PER_CORE = DIM // N_CORES       # 128
TRIPLES = A_PER_CORE * P * K      # 4096 triples per core
N_IDX = 2 * TRIPLES               # 8192 gather indices per core
N_GROUPS = 8                      # Q7 core groups (16 partitions each)
NPG = N_IDX // N_GROUPS           # 1024 gathers per group
TPG = NPG // 2                    # 512 triples per group
TBL_ROWS = K * D1                 # 8200 combined (k, j) rows
NFLD = 3                          # interleaved fields [f0, f1, f2]

_PROGS = {}  # iters -> compiled program (built once per process)


def _build_program(iters=1):
    f32, i16 = mybir.dt.float32, mybir.dt.int16
    alu = mybir.AluOpType
    nc = bacc.Bacc("TRN2")

    tbl = nc.dram_tensor("tbl", [1, TBL_ROWS, NFLD], f32, kind="ExternalInput")
    idx = nc.dram_tensor("idx", [128, NPG // 16], i16, kind="ExternalInput")
    out = nc.dram_tensor("out", [128, 8], f32, kind="ExternalOutput")

    with (
        nc.sbuf_tensor("tbl_sb", [128, TBL_ROWS, NFLD], f32) as tbl_sb,
        nc.sbuf_tensor("idx_sb", [128, NPG // 16], i16) as idx_sb,
        nc.sbuf_tensor("gath", [128, 2, NPG, NFLD], f32) as gath,  # ping-pong
        nc.sbuf_tensor("sneg", [128, TPG], f32) as sneg,
        nc.sbuf_tensor("s2", [128, TPG], f32) as s2,
        nc.sbuf_tensor("gs", [128, TPG], f32) as gs,
        nc.sbuf_tensor("junk", [128, TPG], f32) as junk,
        nc.sbuf_tensor("red", [128, 8], f32) as red,
        nc.semaphore("s_in") as s_in,
        nc.semaphore("s_g") as s_g,
        nc.semaphore("s_v") as s_v,
        nc.semaphore("s_o") as s_o,
        nc.Block() as block,
    ):
        @block.gpsimd
        def _(gp):
            gp.dma_start(tbl_sb[0:1], tbl[:]).then_inc(s_in, 16)
            gp.dma_start(idx_sb[:], idx[:]).then_inc(s_in, 16)
            gp.wait_ge(s_in, 32)
            gp.partition_broadcast(tbl_sb[:], tbl_sb[0:1])
            for i in range(iters):
                if i >= 2:  # ping-pong: consumers of iter i-2 must be done
                    gp.wait_ge(s_v, i - 1)
                gp.ap_gather(
                    gath[:, i % 2], tbl_sb[:], idx_sb[:],
                    channels=128, num_elems=TBL_ROWS, d=NFLD, num_idxs=NPG,
                ).then_inc(s_g, 1)

        @block.vector
        def _(v):
            for i in range(iters):
                b = i % 2
                h0 = gath[:, b, 0:TPG, 0]
                h1 = gath[:, b, TPG:NPG, 0]
                g0 = gath[:, b, 0:TPG, 1]
                g1 = gath[:, b, TPG:NPG, 1]
                a0 = gath[:, b, 0:TPG, 2]
                a1 = gath[:, b, TPG:NPG, 2]
                v.wait_ge(s_g, i + 1)
                # Sneg = f0_0 + f0_1   (T[k] pre-folded into the table)
                v.scalar_tensor_tensor(
                    sneg[:], h0, 0.0, h1, alu.add, alu.add,
                )
                # S2 = Sneg^2 ; red1 = sum S2
                v.scalar_tensor_tensor(
                    s2[:], sneg[:], 0.0, sneg[:], alu.add, alu.mult,
                    accum_out=red[:, 1:2],
                )
                # GS = g0 + g1
                v.scalar_tensor_tensor(
                    gs[:], g0, 0.0, g1, alu.add, alu.add,
                )
                # red2 = sum GS * Sneg^2
                v.scalar_tensor_tensor(
                    junk[:], gs[:], 0.0, s2[:], alu.add, alu.mult,
                    accum_out=red[:, 2:3],
                )
                # red4 = sum (f2_0 + f2_1) = sum(A0+A1) + sum T*Sneg
                v.scalar_tensor_tensor(
                    junk[:], a0, 0.0, a1, alu.add, alu.add,
                    accum_out=red[:, 4:5],
                ).then_inc(s_v, 1)

        @block.sync
        def _(s):
            s.wait_ge(s_v, iters)
            s.dma_start(out[:], red[:]).then_inc(s_o, 16)
            s.wait_ge(s_o, 16)

    nc.finalize()
    return nc


def _get_program(iters=1):
    if iters not in _PROGS:
        _PROGS[iters] = _build_program(iters)
    return _PROGS[iters]


def _device_inputs(meta_theta, m_ks, grads_v, perms):
    """Host prep: replicated field table (O(K*D1)) and per-core index shards."""
    g = np.asarray(grads_v, np.float32)
    c = (np.asarray(meta_theta, np.float32)[None, :] - np.asarray(m_ks, np.float32))
    c = c.astype(np.float32)
    h = (g * c).astype(np.float32)
    lg = (0.5 * np.log(g.astype(np.float64))).astype(np.float32)
    a_f = (lg - np.float32(0.5) * g * c * c).astype(np.float32)
    t_k = h.astype(np.float64).sum(axis=1).astype(np.float32)  # (K,)

    f0 = (h - 0.5 * t_k[:, None]).astype(np.float32)           # h - T/2
    f2 = (a_f + t_k[:, None] * f0).astype(np.float32)          # A + T*f0
    tbl_row = np.empty((TBL_ROWS, NFLD), np.float32)
    tbl_row[:, 0] = f0.ravel()
    tbl_row[:, 1] = g.ravel()
    tbl_row[:, 2] = f2.ravel()
    tbl = np.ascontiguousarray(tbl_row[None])  # [1, TBL_ROWS, NFLD]

    perms01 = np.ascontiguousarray(np.asarray(perms)[:, :, :, :2])  # (A,P,K,2)
    kvec = np.tile(np.arange(K, dtype=np.int64), TRIPLES // K)      # t = (a',p,k)

    in_maps = []
    for core in range(N_CORES):
        sl = perms01[core * A_PER_CORE : (core + 1) * A_PER_CORE]
        sl = sl.reshape(TRIPLES, 2).astype(np.int64)
        comb0 = kvec * D1 + sl[:, 0]
        comb1 = kvec * D1 + sl[:, 1]
        # group g handles triples [512g, 512(g+1)): gathers 0..511 = comb0,
        # 512..1023 = comb1; wrapped (s p) across the group's 16 partitions.
        idxc = np.empty((128, NPG // 16), np.int16)
        for grp in range(N_GROUPS):
            tsl = slice(grp * TPG, (grp + 1) * TPG)
            idx_g = np.concatenate([comb0[tsl], comb1[tsl]]).astype(np.int16)
            idxc[grp * 16 : (grp + 1) * 16] = idx_g.reshape(NPG // 16, 16).T
        in_maps.append({"tbl": tbl, "idx": np.ascontiguousarray(idxc)})
    return in_maps


def _finalize(partials, meta_theta, alpha):
    """Combine per-core partial sums with the constant and prior terms.

    partials: (N_CORES, 128, 8); within each 16-partition group all rows are
    identical, so take one row per group.  Used columns: red1 = sum Sneg^2,
    red2 = sum GS*Sneg^2, red4 = sum(A0+A1) + sum T*Sneg.
    """
    p = np.asarray(partials, np.float64)[:, ::16, :]  # (8, 8 groups, 8)
    total = float(p[:, :, 1].sum() - 0.5 * p[:, :, 2].sum() + p[:, :, 4].sum())
    sum_lp = total - LOG2PI * (N_CORES * TRIPLES)
    loss_pred = sum_lp / (P * M_COND * K)
    mt = np.asarray(meta_theta, np.float64)
    a = float(alpha)
    lp_prior = -0.5 * (D1 * LOG2PI + D1 * np.log(a) + float(mt @ mt) / a)
    loss = (1.0 - 1.0 / K) * lp_prior + loss_pred
    return np.float32(-loss)


def run_device(in_maps, iters=1, **kwargs):
    nc = _get_program(iters)
    return run_bass_kernel_spmd(nc, in_maps, list(range(N_CORES)), **kwargs)


def kernel(meta_theta, m_ks, grads_v, perms, alpha):
    in_maps = _device_inputs(meta_theta, m_ks, grads_v, perms)
    last_err = None
    for _ in range(3):  # retry transient device/runtime hiccups
        try:
            res = run_device(in_maps)
            break
        except Exception as e:  # noqa: BLE001
            last_err = e
    else:
        raise last_err
    partials = np.stack([r["out"] for r in res.results])  # (8, 128, 8)
    return _finalize(partials, meta_theta, alpha)
